# revision 6
# baseline (speedup 1.0000x reference)
"""GAT-style attention layer on 8 TRN2 NeuronCores (bass/Tile).

Sharding: rows (nodes) of x/adj/output are split across 8 cores; weights
replicated.  Per core: h_loc = lrelu(x_loc @ W); AllGather h (bf16) and
wh2 = h @ a2; then for each 128-row block compute
att = adj * exp(lrelu(wh1[m] + wh2[j])) tile-by-tile, transpose att tiles
on the PE and accumulate att @ h_full into PSUM.  The row-softmax
denominator cancels under the subsequent L2 row-normalisation, so it is
never computed.  Epilogue: out = (att@h)/||att@h|| + (x@res_w + bias+res_b).
"""

import numpy as np

ALPHA = 0.2
N_CORES = 8

_CACHE: dict = {}


def build_nc(n_total=8192, d_in=512, d_out=256):
    from concourse import bacc, tile, mybir
    from concourse.masks import make_identity

    f32 = mybir.dt.float32
    bf16 = mybir.dt.bfloat16
    i32 = mybir.dt.int32
    Alu = mybir.AluOpType
    AF = mybir.ActivationFunctionType

    m_loc = n_total // N_CORES            # rows per core
    MT = m_loc // 128                     # m-tiles per core
    KT = d_in // 128                      # contraction tiles for x@W
    JT = n_total // 128                   # j-tiles (global)
    JCH = min(2048, n_total)              # j elementwise chunk
    NCH = n_total // JCH
    TPC = JCH // 128                      # att tiles per chunk
    G = 4                                 # transposes batched per PSUM tile
    NG = TPC // G

    nc = bacc.Bacc("TRN2", target_bir_lowering=False, debug=False,
                   num_devices=N_CORES)

    x_d = nc.dram_tensor("x", [m_loc, d_in], f32, kind="ExternalInput")
    adj_d = nc.dram_tensor("adj", [m_loc, n_total], i32, kind="ExternalInput")
    w_d = nc.dram_tensor("weight", [d_in, d_out], f32, kind="ExternalInput")
    a_d = nc.dram_tensor("a", [1, 2 * d_out], f32, kind="ExternalInput")
    bias_d = nc.dram_tensor("bias", [1, d_out], f32, kind="ExternalInput")
    resw_d = nc.dram_tensor("res_w", [d_in, d_out], f32, kind="ExternalInput")
    resb_d = nc.dram_tensor("res_b", [1, d_out], f32, kind="ExternalInput")
    out_d = nc.dram_tensor("out", [m_loc, d_out], f32, kind="ExternalOutput")

    with tile.TileContext(nc) as tc:
        with (
            tc.tile_pool(name="dram", bufs=1, space="DRAM") as dram,
            tc.tile_pool(name="const", bufs=1) as const,
            tc.tile_pool(name="persist", bufs=1) as persist,
            tc.tile_pool(name="ld", bufs=2) as ld,
            tc.tile_pool(name="work", bufs=2) as work,
            tc.tile_pool(name="attw", bufs=2) as attw,
            tc.tile_pool(name="attT", bufs=3) as attTp,
            tc.tile_pool(name="small", bufs=4) as small,
            tc.tile_pool(name="pacc", bufs=3, space="PSUM") as pacc_pool,
            tc.tile_pool(name="ptp", bufs=3, space="PSUM") as ptp_pool,
        ):
            # ---- collective bounce buffers ----
            hg_in = dram.tile([m_loc, d_out], bf16, name="hg_in")
            hg_out = dram.tile([n_total, d_out], bf16, addr_space="Shared",
                               name="hg_out")
            wg_in = dram.tile([m_loc, 1], f32, name="wg_in")
            wg_out = dram.tile([n_total, 1], f32, addr_space="Shared",
                               name="wg_out")

            # ---- constants ----
            ident = const.tile([128, 128], bf16, name="ident")
            make_identity(nc, ident[:])

            w_bf = const.tile([128, KT, d_out], bf16, name="w_bf")
            rw_bf = const.tile([128, KT, d_out], bf16, name="rw_bf")
            for k in range(KT):
                wtmp = ld.tile([128, d_out], f32, name="wtmp")
                nc.sync.dma_start(wtmp[:], w_d[k * 128:(k + 1) * 128, :])
                nc.vector.tensor_copy(w_bf[:, k, :], wtmp[:])
                rtmp = ld.tile([128, d_out], f32, name="rtmp")
                nc.sync.dma_start(rtmp[:], resw_d[k * 128:(k + 1) * 128, :])
                nc.vector.tensor_copy(rw_bf[:, k, :], rtmp[:])

            # a broadcast across partitions: [128, 2*d_out]
            a_bc = const.tile([128, 2 * d_out], f32, name="a_bc")
            nc.gpsimd.dma_start(a_bc[:], a_d.ap().broadcast_to([128, 2 * d_out]))

            # bias_total row = bias + res_b
            bias_sb = small.tile([1, d_out], f32, name="bias_sb")
            nc.sync.dma_start(bias_sb[:], bias_d[:])
            resb_sb = small.tile([1, d_out], f32, name="resb_sb")
            nc.sync.dma_start(resb_sb[:], resb_d[:])
            bias_tot = const.tile([1, d_out], f32, name="bias_tot")
            nc.vector.tensor_add(bias_tot[:], bias_sb[:], resb_sb[:])
            ones_row = const.tile([1, 128], f32, name="ones_row")
            nc.vector.memset(ones_row[:], 1.0)

            # ---- persistent state ----
            xT = persist.tile([128, KT, m_loc], bf16, name="xT")
            wh1 = persist.tile([128, MT], f32, name="wh1")
            res_sb = persist.tile([128, MT, d_out], f32, name="res_sb")
            h_full = persist.tile([128, JT, d_out], bf16, name="h_full")
            wh2_bc = persist.tile([128, n_total], f32, name="wh2_bc")

            # ---- phase 0: local h, wh1/wh2, residual ----
            for mi in range(MT):
                ms = slice(mi * 128, (mi + 1) * 128)
                xld = ld.tile([128, d_in], f32, name="xld")
                nc.sync.dma_start(xld[:], x_d[ms, :])
                xbf = ld.tile([128, d_in], bf16, name="xbf")
                nc.vector.tensor_copy(xbf[:], xld[:])
                tp = ptp_pool.tile([128, 512], bf16, name="tp", tag="tp")
                for k in range(KT):
                    nc.tensor.transpose(tp[:, k * 128:(k + 1) * 128],
                                        xbf[:, k * 128:(k + 1) * 128], ident[:])
                nc.vector.tensor_copy(
                    xT[:, :, ms],
                    tp[:].rearrange("p (k f) -> p k f", k=KT))

                hp = pacc_pool.tile([128, d_out], f32, name="hp", tag="acc")
                for k in range(KT):
                    nc.tensor.matmul(hp[:], xT[:, k, ms], w_bf[:, k, :],
                                     start=(k == 0), stop=(k == KT - 1))
                hf = work.tile([128, d_out], f32, name="hf")
                nc.scalar.activation(hf[:], hp[:], AF.Lrelu, alpha=ALPHA)
                hb = work.tile([128, d_out], bf16, name="hb")
                nc.vector.tensor_copy(hb[:], hf[:])
                nc.sync.dma_start(hg_in[ms, :], hb[:])

                scr = work.tile([128, d_out], f32, name="scr")
                nc.vector.tensor_mul(scr[:], hf[:], a_bc[:, 0:d_out])
                nc.vector.tensor_reduce(wh1[:, mi:mi + 1], scr[:],
                                        axis=mybir.AxisListType.X, op=Alu.add)
                scr2 = work.tile([128, d_out], f32, name="scr2")
                wh2t = small.tile([128, 1], f32, name="wh2t")
                nc.vector.tensor_mul(scr2[:], hf[:], a_bc[:, d_out:2 * d_out])
                nc.vector.tensor_reduce(wh2t[:], scr2[:],
                                        axis=mybir.AxisListType.X, op=Alu.add)
                nc.sync.dma_start(wg_in[ms, :], wh2t[:])

                rp = pacc_pool.tile([128, d_out], f32, name="rp", tag="acc")
                for k in range(KT):
                    nc.tensor.matmul(rp[:], xT[:, k, ms], rw_bf[:, k, :],
                                     start=(k == 0), stop=False)
                nc.tensor.matmul(rp[:], ones_row[:], bias_tot[:],
                                 start=False, stop=True)
                nc.vector.tensor_copy(res_sb[:, mi, :], rp[:])

            # ---- collectives ----
            rg = [list(range(N_CORES))]
            nc.gpsimd.collective_compute(
                "AllGather", Alu.bypass, replica_groups=rg,
                ins=[hg_in.opt()], outs=[hg_out.opt()])
            nc.gpsimd.collective_compute(
                "AllGather", Alu.bypass, replica_groups=rg,
                ins=[wg_in.opt()], outs=[wg_out.opt()])

            # ---- load gathered h and broadcast wh2 ----
            nc.sync.dma_start(
                h_full[:],
                hg_out[:, :].rearrange("(j p) n -> p j n", p=128))
            nc.gpsimd.dma_start(
                wh2_bc[:],
                wg_out[:, :].rearrange("a b -> b a").broadcast_to(
                    [128, n_total]))

            # ---- main attention loop ----
            for mi in range(MT):
                ms = slice(mi * 128, (mi + 1) * 128)
                pacc = pacc_pool.tile([128, d_out], f32, name="pacc", tag="acc")
                for jc in range(NCH):
                    js = slice(jc * JCH, (jc + 1) * JCH)
                    adj_t = attw.tile([128, JCH], i32, name="adj_t")
                    nc.sync.dma_start(adj_t[:], adj_d[ms, js])
                    lr = attw.tile([128, JCH], f32, name="lr")
                    nc.scalar.activation(lr[:], wh2_bc[:, js], AF.Lrelu,
                                         bias=wh1[:, mi:mi + 1], scale=1.0,
                                         alpha=ALPHA)
                    ex = attw.tile([128, JCH], bf16, name="ex")
                    nc.scalar.activation(ex[:], lr[:], AF.Exp)
                    at = attw.tile([128, JCH], bf16, name="at")
                    nc.vector.tensor_tensor(at[:], ex[:], adj_t[:], Alu.mult)
                    for g in range(NG):
                        tp2 = ptp_pool.tile([128, 512], bf16, name="tp2", tag="tp")
                        for t in range(G):
                            jt = g * G + t
                            nc.tensor.transpose(
                                tp2[:, t * 128:(t + 1) * 128],
                                at[:, jt * 128:(jt + 1) * 128], ident[:])
                        atT = attTp.tile([128, 512], bf16, name="atT")
                        nc.vector.tensor_copy(atT[:], tp2[:])
                        for t in range(G):
                            jglob = jc * TPC + g * G + t
                            nc.tensor.matmul(
                                pacc[:], atT[:, t * 128:(t + 1) * 128],
                                h_full[:, jglob, :],
                                start=(jglob == 0), stop=(jglob == JT - 1))
                # epilogue
                scr3 = work.tile([128, d_out], f32, name="scr3")
                ssq = small.tile([128, 1], f32, name="ssq")
                nc.scalar.activation(scr3[:], pacc[:], AF.Square,
                                     accum_out=ssq[:])
                nrm = small.tile([128, 1], f32, name="nrm")
                nc.scalar.sqrt(nrm[:], ssq[:])
                inv = small.tile([128, 1], f32, name="inv")
                nc.vector.reciprocal(inv[:], nrm[:])
                outt = work.tile([128, d_out], f32, name="outt")
                nc.vector.scalar_tensor_tensor(
                    out=outt[:], in0=pacc[:], scalar=inv[:],
                    in1=res_sb[:, mi, :], op0=Alu.mult, op1=Alu.add)
                nc.sync.dma_start(out_d[ms, :], outt[:])

    nc.compile()
    return nc


def _get_nc(n_total=8192):
    key = ("nc", n_total)
    if key not in _CACHE:
        _CACHE[key] = build_nc(n_total)
    return _CACHE[key]


def make_in_maps(x, adj, weight, a, bias, res_w, res_b):
    n_total = x.shape[0]
    m_loc = n_total // N_CORES
    d_out = weight.shape[1]
    x = np.ascontiguousarray(np.asarray(x, dtype=np.float32))
    adj = np.ascontiguousarray(np.asarray(adj, dtype=np.int32))
    weight = np.ascontiguousarray(np.asarray(weight, dtype=np.float32))
    a_row = np.ascontiguousarray(
        np.asarray(a, dtype=np.float32).reshape(1, 2 * d_out))
    bias_row = np.ascontiguousarray(
        np.asarray(bias, dtype=np.float32).reshape(1, d_out))
    res_w = np.ascontiguousarray(np.asarray(res_w, dtype=np.float32))
    resb_row = np.ascontiguousarray(
        np.asarray(res_b, dtype=np.float32).reshape(1, d_out))
    return [
        {
            "x": x[c * m_loc:(c + 1) * m_loc],
            "adj": adj[c * m_loc:(c + 1) * m_loc],
            "weight": weight,
            "a": a_row,
            "bias": bias_row,
            "res_w": res_w,
            "res_b": resb_row,
        }
        for c in range(N_CORES)
    ]


def _run(nc, in_maps, **kw):
    from concourse import bass_utils
    return bass_utils.run_bass_kernel_spmd(
        nc, in_maps, core_ids=list(range(N_CORES)), **kw)


def kernel(x, adj, weight, a, bias, res_w, res_b):
    n_total = x.shape[0]
    nc = _get_nc(n_total)
    in_maps = make_in_maps(x, adj, weight, a, bias, res_w, res_b)
    res = _run(nc, in_maps)
    return np.concatenate(
        [res.results[c]["out"] for c in range(N_CORES)], axis=0)
